# revision 7
# baseline (speedup 1.0000x reference)
"""CBOW negative-sampling loss kernel for Trainium2 (8 NeuronCores, SPMD).

Strategy
--------
The whole problem is an embedding gather (memory-regime): per batch element b
we need 21 rows of 50 floats (10 context rows from in_embed, 1 pos + 10 neg
rows from out_embed), then a handful of dot products and log-sigmoids, and a
global mean.  Total gather traffic dominates everything else.

Host side:
  * concat the two embedding tables into one [2*VOCAB, 50] table (out_embed
    rows offset by VOCAB) and cast to fp16 (halves gather bytes; error in the
    final scalar is ~1e-5 relative, far below tolerance),
  * build one [B, 21] int32 index tensor: [ctx(10), pos+V, neg+V(10)],
  * shard the batch across 8 cores; per core lay indices out as
    [128 partitions, ntiles, 21] so batch element t*128+p sits on partition p.

Device side (identical program on all 8 cores):
  * one DMA loads the core's whole index block to SBUF,
  * per supertile (T batch-tiles of 128): one indirect DMA gathers
    128*T*21 rows of 50 fp16 values (one descriptor per row),
  * DVE: tree-add the 10 context rows, multiply the 11 out rows by the
    (unnormalized) context sum broadcast, tensor_reduce to 11 raw scores
    (pos score negated via the reduce's negate flag),
  * tail: sigmoid(-0.1*x) then ln(x + 1e-10) with accum_out gives the
    per-partition sum of log-sigmoid terms in one pass; 1/10 context mean
    and the pos/neg sign bookkeeping are folded into the -0.1 scale,
  * DMA the [128,1] partial sum out.

Host finishes: loss = -(sum of all partials) / B.
"""

import os
import sys

import numpy as np

if "/opt/trn_rl_repo" not in sys.path:
    sys.path.insert(0, "/opt/trn_rl_repo")

from concourse import bass, mybir  # noqa: E402
from concourse import bass_utils  # noqa: E402
from concourse import tile  # noqa: E402
from concourse.bacc import Bacc  # noqa: E402

VOCAB = 50000
DIM = 50
B = 131072
CTX = 10
NEG = 10
NIDX = CTX + 1 + NEG  # 21 gathered rows per batch element
EPS = 1e-10

NCORES = 8
P = 128
BC = B // NCORES  # 16384 batch elements per core
NTILES = BC // P  # 128 tiles of 128 elements
T_SUPER = 8  # batch-tiles per gather instruction

_PROFILE = False  # test.py flips this to get a traced run + HW exec time
LAST_EXEC_NS = None


def build_nc(ntiles: int = NTILES, t_super: int = T_SUPER, repeats: int = 1):
    """Build the per-core Bass program.  ntiles must be divisible by t_super.

    repeats > 1 re-runs the whole compute body (used only for timing: the
    per-iteration HW time is the wall-clock delta between two repeat counts).
    """
    assert ntiles % t_super == 0
    nsuper = ntiles // t_super
    fp16 = mybir.dt.float16
    f32 = mybir.dt.float32

    nc = Bacc(None, target_bir_lowering=False)
    # activation(bias=<float>) looks the constant up in const_aps; only 0.0
    # and 1.0 are pre-registered, so register EPS the same way Bass does.
    eps_t = nc.alloc_sbuf_tensor("const-eps", [P, 1], f32)
    nc.gpsimd.memset(eps_t.ap(), EPS)
    nc.const_aps.aps[(f32, EPS)] = eps_t.ap()
    nc.all_engine_barrier()

    table = nc.dram_tensor("table", [2 * VOCAB, DIM], fp16, kind="ExternalInput")
    idx = nc.dram_tensor("idx", [P, ntiles * NIDX], mybir.dt.int32, kind="ExternalInput")
    partial = nc.dram_tensor("partial", [P, 1], f32, kind="ExternalOutput")

    with tile.TileContext(nc) as tc:
        with (
            tc.tile_pool(name="idxp", bufs=1) as ipool,
            tc.tile_pool(name="gather", bufs=3) as gpool,
            tc.tile_pool(name="work", bufs=2) as wpool,
            tc.tile_pool(name="stage", bufs=1) as spool,
        ):
          for rep in range(repeats):
            it = ipool.tile([P, ntiles * NIDX], mybir.dt.int32, tag="it")
            nc.sync.dma_start(out=it[:], in_=idx[:])
            itv = it[:].rearrange("p (t j) -> p t j", t=ntiles, j=NIDX)

            scores = spool.tile([P, ntiles * 11], f32, tag="scores")
            sv = scores[:].rearrange("p (t j) -> p t j", t=ntiles, j=11)

            for s in range(nsuper):
                t0 = s * t_super
                g = gpool.tile([P, t_super * NIDX * DIM], fp16, tag="g")
                nc.gpsimd.indirect_dma_start(
                    out=g[:],
                    out_offset=None,
                    in_=table[:],
                    in_offset=bass.IndirectOffsetOnAxis(
                        ap=itv[:, t0 : t0 + t_super, :], axis=0
                    ),
                )
                g4 = g[:].rearrange(
                    "p (t j d) -> p t j d", t=t_super, j=NIDX, d=DIM
                )
                # context sum over the 10 ctx rows: 10 = (5+5) -> (2+2)+(1) tree
                s1 = wpool.tile([P, t_super * 5 * DIM], fp16, tag="s1")
                s1v = s1[:].rearrange("p (t j d) -> p t j d", t=t_super, j=5, d=DIM)
                nc.vector.tensor_add(
                    out=s1v, in0=g4[:, :, 0:5, :], in1=g4[:, :, 5:10, :]
                )
                s2 = wpool.tile([P, t_super * 2 * DIM], fp16, tag="s2")
                s2v = s2[:].rearrange("p (t j d) -> p t j d", t=t_super, j=2, d=DIM)
                nc.vector.tensor_add(
                    out=s2v, in0=s1v[:, :, 0:2, :], in1=s1v[:, :, 2:4, :]
                )
                s3 = wpool.tile([P, t_super * DIM], fp16, tag="s3")
                s3v = s3[:].rearrange("p (t d) -> p t d", t=t_super, d=DIM)
                nc.vector.tensor_add(
                    out=s3v, in0=s2v[:, :, 0, :], in1=s2v[:, :, 1, :]
                )
                ctx = wpool.tile([P, t_super * DIM], fp16, tag="ctx")
                ctxv = ctx[:].rearrange("p (t d) -> p t d", t=t_super, d=DIM)
                nc.vector.tensor_add(out=ctxv, in0=s3v, in1=s1v[:, :, 4, :])

                # raw scores: (ctx_sum . out_row) for pos + 10 neg rows
                prod = wpool.tile([P, t_super * 11 * DIM], fp16, tag="prod")
                prodv = prod[:].rearrange(
                    "p (t j d) -> p t j d", t=t_super, j=11, d=DIM
                )
                ctxb = ctxv.unsqueeze(2).to_broadcast([P, t_super, 11, DIM])
                nc.vector.tensor_mul(out=prodv, in0=g4[:, :, 10:21, :], in1=ctxb)

                # pos score stored negated so one sigmoid scale works for all
                nc.vector.tensor_reduce(
                    out=sv[:, t0 : t0 + t_super, 0:1],
                    in_=prodv[:, :, 0:1, :],
                    axis=mybir.AxisListType.X,
                    op=mybir.AluOpType.add,
                    negate=True,
                )
                nc.vector.tensor_reduce(
                    out=sv[:, t0 : t0 + t_super, 1:11],
                    in_=prodv[:, :, 1:11, :],
                    axis=mybir.AxisListType.X,
                    op=mybir.AluOpType.add,
                    negate=False,
                )

            # log(sigmoid(score/10) + eps) summed per partition.
            # scores hold [-raw_pos, raw_neg...]; sigma wants (+0.1*raw_pos,
            # -0.1*raw_neg) so scale = -0.1 handles both.
            acc = spool.tile([P, 1], f32, tag="acc")
            nc.scalar.activation(
                out=scores[:],
                in_=scores[:],
                func=mybir.ActivationFunctionType.Sigmoid,
                scale=-0.1,
            )
            nc.scalar.activation(
                out=scores[:],
                in_=scores[:],
                func=mybir.ActivationFunctionType.Ln,
                bias=EPS,
                accum_out=acc[:],
            )
            nc.sync.dma_start(out=partial[:], in_=acc[:])

    nc.compile()
    return nc


def _prep_inputs(context_idxs, pos_target, neg_samples, in_embed_W, out_embed_W):
    idx_all = np.concatenate(
        [
            np.asarray(context_idxs, dtype=np.int64),
            np.asarray(pos_target, dtype=np.int64)[:, None] + VOCAB,
            np.asarray(neg_samples, dtype=np.int64) + VOCAB,
        ],
        axis=1,
    ).astype(np.int32)  # [B, 21]
    table = np.concatenate(
        [np.asarray(in_embed_W), np.asarray(out_embed_W)], axis=0
    ).astype(np.float16)  # [2*VOCAB, 50]

    in_maps = []
    for c in range(NCORES):
        sl = idx_all[c * BC : (c + 1) * BC]  # [BC, 21]
        idx_c = (
            sl.reshape(NTILES, P, NIDX)
            .transpose(1, 0, 2)
            .reshape(P, NTILES * NIDX)
            .copy()
        )
        in_maps.append({"table": table, "idx": idx_c})
    return in_maps


def kernel(context_idxs, pos_target, neg_samples, in_embed_W, out_embed_W):
    in_maps = _prep_inputs(
        context_idxs, pos_target, neg_samples, in_embed_W, out_embed_W
    )
    nc = build_nc()
    res = bass_utils.run_bass_kernel_spmd(nc, in_maps, core_ids=list(range(NCORES)))
    total = sum(float(r["partial"].sum()) for r in res.results)
    return np.float32(-total / B)
